# revision 7
# baseline (speedup 1.0000x reference)
"""Trainium2 Bass kernel: banded-attention transformer encoder layer.

Sharding: 8 cores, data-parallel over batch (2) x sequence-parallel (4).
Each core computes 1024 tokens end-to-end locally (attention needs only a
W-token halo of keys/values, supplied by the host shard). No collectives.

Per-core pipeline (T=1024 local tokens, D=1024, Dff=4096, W=8):
  A. Banded attention in transposed layout: scoresT[k,q] = K^T Q via f32r
     matmuls (N=256 query blocks), additive band mask, exp (no max-sub:
     |s/sqrt(D)| <= ~6 so exp is safe), denominator via ones-column matmul,
     AV token-major with unnormalized probs, normalize on psum eviction.
  B. x = LN1(src + attn) token-major (bn_stats/bn_aggr), then PE-transpose
     x -> xT (bf16, d-major) for the FFN.
  C. FFN1: hT[f,t] = relu(w1 @ x + b1), bf16 matmuls, f-major intermediate.
  D. FFN2: y[t,d] = w2 @ h + b2 token-major (lhsT = hT slices), residual +
     LN2 token-major, DMA out.
"""

import sys

for _p in ("/opt/trn_rl_repo",):
    if _p not in sys.path:
        sys.path.insert(0, _p)

import numpy as np
import ml_dtypes

import concourse.bass as bass
import concourse.mybir as mybir
import concourse.tile as tile
from concourse import bacc
from concourse.bass_utils import run_bass_kernel_spmd
from concourse.masks import make_identity

F32 = mybir.dt.float32
F32R = mybir.dt.float32r
BF16 = mybir.dt.bfloat16

B, S, D, DFF = 2, 4096, 1024, 4096
NCORES = 8
T = (B * S) // NCORES          # 1024 tokens per core
P = 128
NT = T // P                    # 8 token tiles per core
ND = D // P                    # 8 d-chunks
NF = DFF // P                  # 32 f-chunks
QB = 256                       # query block width (matmul N for scoresT)
NB = T // QB                   # 4 query blocks per core
EPS = 1e-5


def _halo_pad(W):
    # keys for block b span halo cols [QB*b, QB*b + QB + 2W) -> chunked to 128
    nkc = -(-(QB + 2 * W) // P)              # chunks per block
    need = QB * (NB - 1) + nkc * P           # last block's chunk end
    return nkc, max(need, ((T + 2 * W + P - 1) // P) * P)


def build(W=8, iters=1, affine=True):
    """Build the per-core Bass program. Returns (nc, input name list)."""
    assert 1 <= W <= 64
    NKC, HALO = _halo_pad(W)
    SCALE = 1.0 / float(np.sqrt(D))

    nc = bacc.Bacc(None, target_bir_lowering=False, debug=False)

    srcT = nc.dram_tensor("srcT", [D, HALO], F32R, kind="ExternalInput")
    srcv = nc.dram_tensor("srcv", [HALO, D], F32R, kind="ExternalInput")
    srcres = nc.dram_tensor("srcres", [T, D], F32, kind="ExternalInput")
    masks = nc.dram_tensor("masks", [NB * NKC, P, QB], F32, kind="ExternalInput")
    w1r = nc.dram_tensor("w1r", [NF, P, ND, P], BF16, kind="ExternalInput")
    w2r = nc.dram_tensor("w2r", [2, NF, P, 512], BF16, kind="ExternalInput")
    b1r = nc.dram_tensor("b1r", [P, NF], F32, kind="ExternalInput")
    gbv = nc.dram_tensor("gbv", [5, D], F32, kind="ExternalInput")
    out = nc.dram_tensor("out", [T, D], F32, kind="ExternalOutput")


    with tile.TileContext(nc) as tc:
        with tc.tile_pool(name="const", bufs=1) as const, \
             tc.tile_pool(name="xpool", bufs=1) as xpool, \
             tc.tile_pool(name="xTpool", bufs=1) as xTpool, \
             tc.tile_pool(name="stats", bufs=1) as stats, \
             tc.tile_pool(name="psA", bufs=4, space="PSUM") as psA, \
             tc.tile_pool(name="psB", bufs=4, space="PSUM") as psB:

            eps_t = const.tile([P, 1], F32, name="eps_t")
            nc.vector.memset(eps_t[:], EPS)
            zero_t = const.tile([P, 1], F32, name="zero_t")
            nc.vector.memset(zero_t[:], 0.0)
            ones32 = const.tile([P, 2], F32, name="ones32")
            nc.vector.memset(ones32[:], 1.0)
            ones_r = const.tile([P, 2], F32R, name="ones_r")
            nc.scalar.copy(out=ones_r[:], in_=ones32[:])
            ident = const.tile([P, P], F32, name="ident")
            make_identity(nc, ident[:])
            b1sb = const.tile([P, NF], F32, name="b1sb")
            nc.sync.dma_start(out=b1sb[:], in_=b1r[:])
            if affine:
                gb = const.tile([P, 5, D], F32, name="gb")
                h = gbv[:]
                nc.sync.dma_start(out=gb[:], in_=bass.AP(
                    tensor=h.tensor, offset=h.offset,
                    ap=[[0, P], h.ap[0], h.ap[1]]))
                g1b, be1b, g2b, be2b, b2b = (gb[:, i, :] for i in range(5))
            mks = const.tile([P, NB * NKC, QB], F32, name="mks")
            nc.sync.dma_start(out=mks[:], in_=masks.rearrange(
                "m p q -> p m q"))

            xs = [xpool.tile([P, D], F32, name=f"x{t}") for t in range(NT)]
            mv1 = [stats.tile([P, 2], F32, name=f"mv1_{t}") for t in range(NT)]
            mv2 = [stats.tile([P, 2], F32, name=f"mv2_{t}") for t in range(NT)]
            varg1 = stats.tile([P, NT], F32, name="varg1")
            rstd1 = stats.tile([P, NT], F32, name="rstd1")
            varg2 = stats.tile([P, NT], F32, name="varg2")
            rstd2 = stats.tile([P, NT], F32, name="rstd2")
            xT = [xTpool.tile([P, T], BF16, name=f"xT{dc}")
                  for dc in range(ND)]

            for _ in range(iters):
                # ---------------- Phase A: attention + residual ----------
                with tc.tile_pool(name="sTp", bufs=2) as sTp, \
                     tc.tile_pool(name="vp", bufs=2) as vp, \
                     tc.tile_pool(name="resp", bufs=2) as resp, \
                     tc.tile_pool(name="tmpp", bufs=3) as tmpp, \
                     tc.tile_pool(name="attp", bufs=2) as attp, \
                     tc.tile_pool(name="Ep", bufs=2) as Ep:
                    for b in range(NB):
                        c0 = QB * b           # halo col of first key chunk
                        sT = sTp.tile([P, ND, NKC * P], F32R, tag="sT",
                                      name=f"sT{b}")
                        nc.sync.dma_start(
                            out=sT[:],
                            in_=srcT.rearrange("(dc p) h -> p dc h", p=P)[
                                :, :, c0:c0 + NKC * P])
                        Es = []
                        for kc in range(NKC):
                            sps = psA.tile([P, QB], F32, tag="psA",
                                           name=f"sc{b}_{kc}")
                            for dc in range(ND):
                                nc.tensor.matmul(
                                    sps[:],
                                    sT[:, dc, P * kc:P * (kc + 1)],
                                    sT[:, dc, W:W + QB],
                                    start=(dc == 0), stop=(dc == ND - 1))
                            tmp = tmpp.tile([P, QB], F32, tag="tmp",
                                            name=f"tmp{b}_{kc}")
                            nc.vector.tensor_add(tmp[:], sps[:],
                                                 mks[:, NKC * b + kc, :])
                            E = Ep.tile([P, QB], F32R, tag=f"E{kc}",
                                        name=f"E{b}_{kc}")
                            nc.scalar.activation(
                                E[:], tmp[:],
                                mybir.ActivationFunctionType.Exp, scale=SCALE)
                            Es.append(E)
                        vt = vp.tile([P, NKC, D], F32R, tag="v",
                                     name=f"v{b}")
                        nc.sync.dma_start(
                            out=vt[:],
                            in_=srcv[c0:c0 + NKC * P, :].rearrange(
                                "(j p) d -> p j d", p=P))
                        vs = [vt[:, j, :] for j in range(NKC)]
                        for h in range(QB // P):   # 2 query tiles per block
                            t = (QB // P) * b + h
                            qs = slice(P * h, P * (h + 1))
                            den = psA.tile([P, 2], F32, tag="psA",
                                           name=f"den{t}")
                            nc.tensor.matmul(den[:], Es[h][:, qs],
                                             ones_r[:], start=True, stop=False)
                            nc.tensor.matmul(den[:], Es[h + 1][0:2 * W, qs],
                                             ones_r[0:2 * W, :],
                                             start=False, stop=True)
                            rinv = tmpp.tile([P, 1], F32, tag="rinv",
                                             name=f"rinv{t}")
                            nc.vector.reciprocal(rinv[:], den[:, 0:1])
                            att = attp.tile([P, D], F32, tag="att",
                                            name=f"att{t}")
                            for dh in range(2):
                                ds_ = slice(512 * dh, 512 * (dh + 1))
                                avp = psB.tile([P, 512], F32, tag="psB",
                                               name=f"av{t}_{dh}")
                                nc.tensor.matmul(avp[:], Es[h][:, qs],
                                                 vs[h][:, ds_],
                                                 start=True, stop=False)
                                nc.tensor.matmul(avp[:], Es[h + 1][0:2 * W, qs],
                                                 vs[h + 1][0:2 * W, ds_],
                                                 start=False, stop=True)
                                nc.scalar.activation(
                                    att[:, ds_], avp[:],
                                    mybir.ActivationFunctionType.Copy,
                                    scale=rinv[:])
                            if h == 0:
                                rst = resp.tile([P, 2, D], F32, tag="rs",
                                                name=f"rs{b}")
                                nc.sync.dma_start(
                                    out=rst[:],
                                    in_=srcres[QB * b:QB * (b + 1), :].rearrange(
                                        "(j p) d -> p j d", p=P))
                            nc.vector.tensor_add(xs[t][:], att[:], rst[:, h, :])
                            st1 = tmpp.tile([P, 2, 6], F32, tag="st",
                                            name=f"st1_{t}")
                            for sg in range(2):
                                nc.vector.bn_stats(
                                    st1[:, sg, :],
                                    xs[t][:, 512 * sg:512 * (sg + 1)])
                            nc.vector.bn_aggr(mv1[t][:], st1[:])

                    # keep all Exp ACT ops before the first Sqrt (table sets)
                    tc.no_sync_barrier()

                    # ---------------- LN1 finalize -----------------------
                    for t in range(NT):
                        nc.gpsimd.tensor_copy(out=varg1[:, t:t + 1],
                                              in_=mv1[t][:, 1:2])
                    nc.scalar.activation(varg1[:], varg1[:],
                                         mybir.ActivationFunctionType.Sqrt,
                                         bias=eps_t[:])
                    nc.vector.reciprocal(rstd1[:], varg1[:])
                    for t in range(NT):
                        nc.vector.tensor_scalar(
                            out=xs[t][:], in0=xs[t][:],
                            scalar1=mv1[t][:, 0:1], scalar2=rstd1[:, t:t + 1],
                            op0=mybir.AluOpType.subtract,
                            op1=mybir.AluOpType.mult)
                        if affine:
                            nc.vector.tensor_mul(xs[t][:], xs[t][:], g1b)
                            nc.vector.tensor_add(xs[t][:], xs[t][:], be1b)

                # ---------------- Phase B: transpose x -> xT (bf16) ------
                for t in range(NT):
                    for dc in range(ND):
                        trp = psB.tile([P, P], F32, tag="psB",
                                       name=f"tr{t}_{dc}")
                        nc.tensor.transpose(trp[:], xs[t][:, P * dc:P * (dc + 1)],
                                            ident[:])
                        nc.scalar.activation(
                            xT[dc][:, P * t:P * (t + 1)], trp[:],
                            mybir.ActivationFunctionType.Copy)

                # ---------------- Phase C: FFN1 (bf16) -------------------
                with tc.tile_pool(name="w1p", bufs=3) as w1p, \
                     tc.tile_pool(name="hTp", bufs=1) as hTp, \
                     tc.tile_pool(name="w2p", bufs=1) as w2p, \
                     tc.tile_pool(name="dpool", bufs=1) as dpool, \
                     tc.tile_pool(name="t2p", bufs=3) as t2p, \
                     tc.tile_pool(name="outp", bufs=2) as outp:
                    hT = [hTp.tile([P, T], BF16, name=f"hT{fc}")
                          for fc in range(NF)]
                    for g in range(NF // 4):
                        w1t = w1p.tile([P, 4, ND, P], BF16, tag="w1",
                                       name=f"w1t{g}")
                        nc.sync.dma_start(
                            out=w1t[:],
                            in_=w1r[4 * g:4 * (g + 1)].rearrange(
                                "g p dc f -> p g dc f"))
                        for fi in range(4):
                            fc = 4 * g + fi
                            for tb in range(2):
                                ts_ = slice(512 * tb, 512 * (tb + 1))
                                hps = psB.tile([P, 512], F32, tag="psB",
                                               name=f"h{fc}_{tb}")
                                for dc in range(ND):
                                    nc.tensor.matmul(hps[:], w1t[:, fi, dc, :],
                                                     xT[dc][:, ts_],
                                                     start=(dc == 0),
                                                     stop=(dc == ND - 1))
                                if fc % 2 == 0:
                                    nc.scalar.activation(
                                        hT[fc][:, ts_], hps[:],
                                        mybir.ActivationFunctionType.Relu,
                                        bias=b1sb[:, fc:fc + 1])
                                else:
                                    nc.vector.tensor_scalar(
                                        out=hT[fc][:, ts_], in0=hps[:],
                                        scalar1=b1sb[:, fc:fc + 1],
                                        scalar2=zero_t[:],
                                        op0=mybir.AluOpType.add,
                                        op1=mybir.AluOpType.max)

                    # ------------- Phase D: FFN2 + residual + LN2 --------
                    for dh in range(2):
                        ds_ = slice(512 * dh, 512 * (dh + 1))
                        w2ts = []
                        for g in range(NF // 4):
                            w2t = w2p.tile([P, 4, 512], BF16, tag=f"w2_{g}",
                                           name=f"w2t{g}")
                            nc.sync.dma_start(
                                out=w2t[:],
                                in_=w2r[dh, 4 * g:4 * (g + 1)].rearrange(
                                    "g p j -> p g j"))
                            w2ts.extend(w2t[:, i, :] for i in range(4))
                        for t in range(NT):
                            yps = psB.tile([P, 512], F32, tag="psB",
                                           name=f"y{t}_{dh}")
                            for fc in range(NF):
                                nc.tensor.matmul(yps[:],
                                                 hT[fc][:, P * t:P * (t + 1)],
                                                 w2ts[fc],
                                                 start=(fc == 0),
                                                 stop=(fc == NF - 1))
                            if affine:
                                tmp2 = t2p.tile([P, 512], F32, tag="tmp2",
                                                name=f"tmp2_{t}_{dh}")
                                nc.vector.tensor_add(tmp2[:], yps[:],
                                                     b2b[:, ds_])
                                nc.vector.tensor_add(xs[t][:, ds_],
                                                     xs[t][:, ds_], tmp2[:])
                            else:
                                nc.vector.tensor_add(xs[t][:, ds_], yps[:],
                                                     xs[t][:, ds_])
                            if dh == 1:
                                st2 = t2p.tile([P, 2, 6], F32, tag="st2",
                                               name=f"st2_{t}")
                                for sg in range(2):
                                    nc.vector.bn_stats(
                                        st2[:, sg, :],
                                        xs[t][:, 512 * sg:512 * (sg + 1)])
                                nc.vector.bn_aggr(mv2[t][:], st2[:])

                    for t in range(NT):
                        nc.gpsimd.tensor_copy(out=varg2[:, t:t + 1],
                                              in_=mv2[t][:, 1:2])
                    nc.scalar.activation(varg2[:], varg2[:],
                                         mybir.ActivationFunctionType.Sqrt,
                                         bias=eps_t[:])
                    nc.vector.reciprocal(rstd2[:], varg2[:])
                    for t in range(NT):
                        nc.vector.tensor_scalar(
                            out=xs[t][:], in0=xs[t][:],
                            scalar1=mv2[t][:, 0:1], scalar2=rstd2[:, t:t + 1],
                            op0=mybir.AluOpType.subtract,
                            op1=mybir.AluOpType.mult)
                        if affine:
                            nc.vector.tensor_mul(xs[t][:], xs[t][:], g2b)
                            nc.vector.tensor_add(xs[t][:], xs[t][:], be2b)
                        nc.gpsimd.dma_start(out=out[P * t:P * (t + 1), :],
                                            in_=xs[t][:])

    nc.compile()
    return nc


def make_inputs(src, w1, b1, w2, b2, g1, be1, g2, be2, W):
    """Build per-core in_maps (list of 8 dicts) from full inputs."""
    NKC, HALO = _halo_pad(W)
    src = np.asarray(src, np.float32)
    w1rr = np.ascontiguousarray(
        w1.reshape(NF, P, ND, P).transpose(0, 3, 2, 1)).astype(
            ml_dtypes.bfloat16)
    w2rr = np.ascontiguousarray(
        w2.T.reshape(NF, P, 2, 512).transpose(2, 0, 1, 3)).astype(
            ml_dtypes.bfloat16)
    b1rr = np.ascontiguousarray(b1.reshape(NF, P).T).astype(np.float32)
    gb = np.ascontiguousarray(np.stack(
        [g1, be1, g2, be2, b2]).astype(np.float32))
    shared = {"w1r": w1rr, "w2r": w2rr, "b1r": b1rr, "gbv": gb}
    in_maps = []
    for c in range(NCORES):
        bb, q = divmod(c, S // T)
        s0 = q * T
        halo_tok = np.zeros((HALO, D), np.float32)
        lo, hi = max(0, s0 - W), min(S, s0 + T + W)
        halo_tok[lo - s0 + W: hi - s0 + W] = src[bb, lo:hi]
        srcT_c = np.ascontiguousarray(halo_tok.T)          # [D, HALO]
        # masks[b*NKC+kc, kr, qq]: key halo idx = QB*b + 128*kc + kr
        kh = (QB * np.arange(NB)[:, None, None]
              + P * np.arange(NKC)[None, :, None]
              + np.arange(P)[None, None, :])               # [NB, NKC, P]
        gk = s0 - W + kh                                   # global key pos
        gq = (s0 + QB * np.arange(NB)[:, None, None, None]
              + np.arange(QB)[None, None, None, :])        # [NB,1,1,QB]
        valid = (np.abs(gq - gk[..., None]) <= W) & (gk[..., None] >= 0) \
            & (gk[..., None] < S)
        mk = np.where(valid, np.float32(0.0), np.float32(-3e10))
        mk = mk.reshape(NB * NKC, P, QB).astype(np.float32)
        in_maps.append({
            "srcT": srcT_c, "srcv": halo_tok,
            "srcres": np.ascontiguousarray(src[bb, s0:s0 + T]),
            "masks": np.ascontiguousarray(mk), **shared,
        })
    return in_maps


_BUILD_CACHE = {}


def kernel(src, w1, b1, w2, b2, g1, be1, g2, be2, window_size):
    W = int(np.asarray(window_size))
    affine = not (np.all(g1 == 1.0) and np.all(be1 == 0.0)
                  and np.all(g2 == 1.0) and np.all(be2 == 0.0)
                  and np.all(b2 == 0.0))
    key = (W, affine)
    if key not in _BUILD_CACHE:
        _BUILD_CACHE[key] = build(W, affine=affine)
    nc = _BUILD_CACHE[key]
    in_maps = make_inputs(src, w1, b1, w2, b2, g1, be1, g2, be2, W)
    res = run_bass_kernel_spmd(nc, in_maps, core_ids=list(range(NCORES)))
    outf = np.empty((B, S, D), np.float32)
    for c in range(NCORES):
        bb, q = divmod(c, S // T)
        outf[bb, q * T:(q + 1) * T] = res.results[c]["out"]
    return outf
